# revision 33
# baseline (speedup 1.0000x reference)
"""Trainium2 Bass kernel for DynamicEdgeConstruction (top-k masked softmax
attention matrix).

Computes, for x [B=4, N=4096, C=256], W_q/W_k [256, 64]:
    Q = x @ W_q; K = x @ W_k
    S = Q K^T / sqrt(64)           [B, N, N]
    A = softmax over the top-k entries of each row of S, zeros elsewhere.

Sharding: 8 NeuronCores, 2 per batch element, each handling 2048 query rows
(row-wise sequence parallel; K replicated per batch).

Device algorithm per core:
  - Projections KT/QT on PE in exact fp32 (4 cyc/row), then split into
    fp16 hi/lo pairs (hi = fp16(v), lo = fp16(v - hi)) so the big S matmul
    can run at fp16 speed (1 cyc/row) while keeping ~1e-6 absolute accuracy:
        S ~= Qhi.Khi + Qlo.Khi + Qhi.Klo       (drop Qlo.Klo ~ 1e-7)
    The first two terms come from ONE contraction-128 matmul with stacked
    lhsT = [Qhi; Qlo] against rhs = [Khi; Khi] (Khi duplicated across both
    partition halves via a cheap SBUF->SBUF DMA); the third is a
    contraction-64 matmul against Klo.  2 matmuls per 512 chunk.
  - E = exp(S/8) from PSUM ([128,1024] quarter tiles) straight to SBUF (no
    max subtraction needed: |S/8| <~ 6, well in fp32 exp range); draining
    per-quarter keeps PE fed with free PSUM banks.
  - DVE max8 on E gives the exact top-8 values F8 (fp32, bit-exact compare
    image).  Z = sum(F8[:k]), r = 1/Z, threshold th = F8[k-1].
  - Final A built per column range (GpSimd cannot run tensor_scalar class
    ops - ISA check - so the balance is):
      [0:ACT_COLS)   DVE STT maskE=(E>=th)*E,  ACT Copy-with-scale: *r
      [ACT_COLS:N)   DVE 2x ts maskr=(E>=th)*r, then A = E*maskr as
                     tensor_tensor on DVE (TT_DVE cols) / GpSimd (rest)
  - DMA the fp16 tile out; host upcasts to fp32 while assembling.

  Pipeline: warmup matmuls hold the PE p-state while inputs stream; the
  2-bank projection PSUM pool coexists with the 6-bank S pool so tile 0
  starts right behind the projections; per-iteration emission order is
  stage_b/c(t-1) before stage_a(t) to keep DVE's in-order stream hot; the
  last two tiles shift work off GpSimd and split the final DMA to cut the
  drain tail.
"""

import numpy as np

B, N, C, DK = 4, 4096, 256, 64
NCORES = 8
RPC = N // 2          # rows per core (2048)
P = 128               # partitions
NT = RPC // P         # row tiles per core (16)
CHUNK = 512           # matmul free-dim chunk (one PSUM bank fp32)
HALF = 2048           # S half-tile free size (4 PSUM banks)

ACT_COLS = 576
TT_DVE = 0
# last-two-tiles split (tail latency): more DVE/ACT, less GpSimd
ACT_COLS_L = 1024
TT_DVE_L = 1536
WARMUP_MM = 16        # tiny PE matmuls to hold p-state while inputs stream in

_cache = {}


def _build(k: int, bench_reps: int | None = None):
    """Build + compile the SPMD Bass program for top-k = k (1..8)."""
    import concourse.bass as bass
    import concourse.bacc as bacc
    import concourse.tile as tile
    import concourse.mybir as mybir
    from contextlib import ExitStack

    f32 = mybir.dt.float32
    f16 = mybir.dt.float16
    Alu = mybir.AluOpType
    Act = mybir.ActivationFunctionType

    nc = bacc.Bacc("TRN2", target_bir_lowering=False, debug=False,
                   num_devices=NCORES)

    xT_d = nc.dram_tensor("xT", [C, N], f32, kind="ExternalInput").ap()
    wq_d = nc.dram_tensor("wq", [C, DK], f32, kind="ExternalInput").ap()
    wk_d = nc.dram_tensor("wk", [C, DK], f32, kind="ExternalInput").ap()
    out_d = nc.dram_tensor("out", [RPC, N], f16, kind="ExternalOutput").ap()

    QOFF = 0  # per-core query-column offset into xT is handled host-side

    with tile.TileContext(nc) as tc:
        with ExitStack() as ctx:
            const = ctx.enter_context(tc.tile_pool(name="const", bufs=1))

            xT = [const.tile([P, N], f32, tag=f"xT{i}", name=f"xT{i}")
                  for i in range(2)]
            wq = [const.tile([P, DK], f32, tag=f"wq{i}", name=f"wq{i}")
                  for i in range(2)]
            wk = [const.tile([P, DK], f32, tag=f"wk{i}", name=f"wk{i}")
                  for i in range(2)]
            # Khi stacked twice (partitions 0-63 and 64-127)
            KThh = const.tile([P, N], f16, tag="KThh")
            KTlo = const.tile([DK, N], f16, tag="KTlo")
            # [Qhi; Qlo] stacked
            QThl = const.tile([P, RPC], f16, tag="QThl")

            # inputs: weights first (small), then x by column chunks so the
            # first projection matmul can start after ~1.5us
            nc.sync.dma_start(wk[0][:], wk_d[0:P, :])
            nc.gpsimd.dma_start(wk[1][:], wk_d[P:2 * P, :])
            nc.gpsimd.dma_start(wq[0][:], wq_d[0:P, :])
            nc.sync.dma_start(wq[1][:], wq_d[P:2 * P, :])
            NCH = N // CHUNK
            for ch in range(NCH):
                sl = slice(ch * CHUNK, (ch + 1) * CHUNK)
                nc.sync.dma_start(xT[0][:, sl], xT_d[0:P, sl])
                nc.gpsimd.dma_start(xT[1][:, sl], xT_d[P:2 * P, sl])

            # Projection of one 512-column chunk: PE fp32 (exact), then
            # split the PSUM result into fp16 hi (ACT copy) + lo (DVE sub).
            def proj_chunk(pool, w, sl, hi_dst, lo_dst, xsl=None):
                xsl = xsl if xsl is not None else sl
                pt = pool.tile([DK, CHUNK], f32, tag="sps", name="pt")
                nc.tensor.matmul(out=pt[:], lhsT=w[0][:], rhs=xT[0][:, xsl],
                                 start=True, stop=False)
                nc.tensor.matmul(out=pt[:], lhsT=w[1][:], rhs=xT[1][:, xsl],
                                 start=False, stop=True)
                nc.scalar.copy(hi_dst, pt[:])
                nc.vector.tensor_tensor(lo_dst, pt[:], hi_dst,
                                        op=Alu.subtract)

            def kt_chunk(pool, ch):
                sl = slice(ch * CHUNK, (ch + 1) * CHUNK)
                proj_chunk(pool, wk, sl, KThh[0:DK, sl], KTlo[:, sl])
                # duplicate Khi into partitions 64..127 (cheap SBUF DMA)
                nc.sync.dma_start(KThh[DK:P, sl], KThh[0:DK, sl])

            def qt_chunk(pool, ch):
                sl = slice(ch * CHUNK, (ch + 1) * CHUNK)
                xsl = slice(QOFF + ch * CHUNK, QOFF + (ch + 1) * CHUNK)
                proj_chunk(pool, wq, sl, QThl[0:DK, sl], QThl[DK:P, sl],
                           xsl=xsl)

            # proj PSUM pool (2 banks) coexists with the S pool (6 banks)
            pps = ctx.enter_context(tc.tile_pool(name="proj_ps", bufs=2,
                                                 space="PSUM"))
            # tiny warmup matmuls keep the PE p-state ramped while the
            # first xT chunks stream in, so projections run at full clock
            wsrc = const.tile([DK, DK], f32, tag="wsrc")
            nc.vector.memset(wsrc[:], 0.0)
            wp = pps.tile([DK, DK], f32, tag="sps", name="wp")
            for _ in range(WARMUP_MM):
                nc.tensor.matmul(out=wp[:], lhsT=wsrc[:], rhs=wsrc[:],
                                 start=True, stop=True)
            for ch in range(NCH):
                kt_chunk(pps, ch)
                if ch < RPC // CHUNK:
                    qt_chunk(pps, ch)

            spool = ctx.enter_context(tc.tile_pool(name="esb", bufs=5))
            mpool = ctx.enter_context(tc.tile_pool(name="mask", bufs=3))
            apool = ctx.enter_context(tc.tile_pool(name="aout", bufs=3))
            small = ctx.enter_context(tc.tile_pool(name="small", bufs=4))
            sps = ctx.enter_context(tc.tile_pool(name="sps", bufs=3,
                                                 space="PSUM"))

            state = {}
            QCHUNK_TILES = CHUNK // P   # S-tiles covered per QT chunk (4)
            SCALE = float(DK) ** -0.5   # 1/8, folded into exp's scale

            QTR = 1024            # S quarter-tile (2 PSUM banks)

            def s_quarter(t, q, E, F32c):
                lhs_hl = QThl[:, t * P:(t + 1) * P]        # [128,128]
                lhs_hi = QThl[0:DK, t * P:(t + 1) * P]     # [64,128]
                ps = sps.tile([P, QTR], f32, tag="sps", name=f"ps{q}")
                for ch in range(QTR // CHUNK):
                    psl = slice(ch * CHUNK, (ch + 1) * CHUNK)
                    ksl = slice(q * QTR + ch * CHUNK,
                                q * QTR + (ch + 1) * CHUNK)
                    nc.tensor.matmul(out=ps[:, psl], lhsT=lhs_hl,
                                     rhs=KThh[:, ksl],
                                     start=True, stop=False)
                    nc.tensor.matmul(out=ps[:, psl], lhsT=lhs_hi,
                                     rhs=KTlo[:, ksl],
                                     start=False, stop=True)
                nc.scalar.activation(E[:, q * QTR:(q + 1) * QTR], ps[:],
                                     Act.Exp, bias=0.0, scale=SCALE)
                if F32c is not None:
                    # incremental per-quarter max8 shortens the fill chain
                    nc.vector.max(F32c[:, 8 * q:8 * q + 8],
                                  E[:, q * QTR:(q + 1) * QTR])

            def finish_a(t, E, F32c):
                F8 = small.tile([P, 8], f32, tag="F8", name="F8")
                Z = small.tile([P, 1], f32, tag="Z", name="Z")
                r = small.tile([P, 1], f32, tag="r", name="r")
                if F32c is not None:
                    nc.vector.max(F8[:], F32c[:])
                else:
                    nc.vector.max(F8[:], E[:])
                nc.vector.tensor_reduce(Z[:], F8[:, 0:k],
                                        mybir.AxisListType.X, Alu.add)
                nc.vector.reciprocal(r[:], Z[:])
                state[t] = (E, F8, r)

            def stage_a(t):
                # S in [128,1024] PSUM quarters; exp drains each quarter so
                # matmuls for the next one never wait long on banks
                E = spool.tile([P, N], f32, tag="E", name="E")
                F32c = (small.tile([P, 32], f32, tag="F32c", name="F32c")
                        if t <= 1 else None)
                for q in range(N // QTR):
                    s_quarter(t, q, E, F32c)
                finish_a(t, E, F32c)

            def split_of(t):
                if t >= NT - 2:
                    return ACT_COLS_L, TT_DVE_L
                return ACT_COLS, TT_DVE

            def stage_b(t):
                # maskr = (E >= th) * r  -> fp16  (one DVE 2x tensor_scalar
                # over the TT ranges); maskE = (E >= th) * E for the ACT range
                E, F8, r = state[t]
                ac, _ = split_of(t)
                th = F8[:, k - 1:k]
                maskr = mpool.tile([P, N], f16, tag="mE", name="mE")
                def mask_stt():
                    if ac > 0:
                        nc.vector.scalar_tensor_tensor(
                            maskr[:, 0:ac], E[:, 0:ac], th,
                            E[:, 0:ac], op0=Alu.is_ge, op1=Alu.mult)
                def mask_ts():
                    nc.vector.tensor_scalar(maskr[:, ac:N],
                                            E[:, ac:N], th, r[:, 0:1],
                                            op0=Alu.is_ge, op1=Alu.mult)
                if t < 2:
                    mask_stt(); mask_ts()
                else:
                    mask_ts(); mask_stt()
                state[t] = (E, F8, r, maskr)

            def stage_c(t):
                # A = E * maskr (TT ranges) / maskE * r (ACT range)
                E, _F8, r, maskr = state.pop(t)
                ac, td = split_of(t)
                A = apool.tile([P, N], f16, tag="A", name="A")
                c0, c1 = ac, ac + td
                if ac > 0:
                    nc.scalar.mul(A[:, 0:c0], maskr[:, 0:c0], r[:, 0:1])
                if td > 0:
                    nc.vector.tensor_tensor(A[:, c0:c1], E[:, c0:c1],
                                            maskr[:, c0:c1], op=Alu.mult)
                nc.gpsimd.tensor_tensor(A[:, c1:N], E[:, c1:N],
                                        maskr[:, c1:N], op=Alu.mult)
                if t == NT - 1:
                    # ship the ACT/DVE ranges while GpSimd finishes its TT
                    nc.sync.dma_start(out_d[t * P:(t + 1) * P, 0:c1],
                                      A[:, 0:c1])
                    nc.sync.dma_start(out_d[t * P:(t + 1) * P, c1:N],
                                      A[:, c1:N])
                else:
                    nc.sync.dma_start(out_d[t * P:(t + 1) * P, :], A[:])

            def main_loop():
                for t in range(NT + 1):
                    if t >= 1:
                        stage_b(t - 1)
                        stage_c(t - 1)
                    if t < NT:
                        stage_a(t)

            if bench_reps is None:
                main_loop()
            else:
                with tc.For_i(0, bench_reps, 1):
                    main_loop()

    nc.compile()
    return nc


def _get_program(k: int):
    if k not in _cache:
        _cache[k] = _build(k)
    return _cache[k]


def kernel(x, W_q, W_k, top_k):
    from concourse.bass_utils import run_bass_kernel_spmd

    x = np.asarray(x, dtype=np.float32)
    W_q = np.asarray(W_q, dtype=np.float32)
    W_k = np.asarray(W_k, dtype=np.float32)
    k = int(np.asarray(top_k))
    assert x.shape == (B, N, C) and W_q.shape == (C, DK) and W_k.shape == (C, DK)
    assert 1 <= k <= 8, f"top_k={k} unsupported"

    nc = _get_program(k)

    wq_c = np.ascontiguousarray(W_q, dtype=np.float32)
    wk_c = np.ascontiguousarray(W_k, dtype=np.float32)

    in_maps = []
    for c in range(NCORES):
        b, half = c // 2, c % 2
        xT = np.ascontiguousarray(x[b].T)                      # [C, N]
        if half == 1:
            # roll so this core's query columns sit at offset 0
            xT = np.ascontiguousarray(np.roll(xT, -RPC, axis=1))
        in_maps.append({"xT": xT, "wq": wq_c, "wk": wk_c})

    res = run_bass_kernel_spmd(nc, in_maps, list(range(NCORES)))

    A = np.empty((B, N, N), dtype=np.float32)
    for c in range(NCORES):
        b, half = c // 2, c % 2
        o = res.results[c]["out"].astype(np.float32)           # [RPC, N]
        if half == 1:
            o = np.roll(o, RPC, axis=1)
        A[b, half * RPC:(half + 1) * RPC, :] = o
    return A
